# revision 30
# baseline (speedup 1.0000x reference)
"""DeformF2F (8-layer offset-conv + deformable-conv + relu) on 8 Trainium2 cores.

Data-parallel over batch (image n -> core n). Per-core, all-bf16 data path with
fp32 PSUM accumulation and fp32 index math:

  activations: channels-first padded [C, 40*80] bf16 (halo y=4/x=8, zeros) for
  conv shifts, plus a channels-last striped copy [128, 25*C] (spatial row r at
  partition r%128, stripe r//128) that feeds the transposing SBUF-source
  dma_gather (output lands channels-first).

  per layer: offset conv (matmul-accum) -> DVE index/bilinear-weight math ->
  int16 corner indices wrapped into the gather's 16-partition layout via a DRAM
  round trip (also replicating the 4-corner interleave) -> one dma_gather per
  tap fetching all 4 bilinear corners -> bilinear weights replicated across
  partitions by a stride-0 DRAM read -> DVE mul/add interp -> per-tap matmul
  accumulation into PSUM -> ACT eviction fused with bias+relu (+bf16 cast) ->
  dma_start_transpose rebuilds the striped gather source for the next layer.
"""
import numpy as np
import sys

sys.path.insert(0, "/opt/trn_rl_repo")

import concourse.bass as bass
import concourse.bacc as bacc
import concourse.mybir as mybir
import concourse.tile as tile
from concourse.bass import AP
from concourse import bass_utils

from concourse import library_config
import ml_dtypes

BF16 = ml_dtypes.bfloat16
FP32 = mybir.dt.float32
BF = mybir.dt.bfloat16
I16 = mybir.dt.int16
AF = mybir.ActivationFunctionType
OP = mybir.AluOpType

H, W = 32, 64
HY, HX = 4, 8
R, Cw = H + 2 * HY, W + 2 * HX     # 40, 80
S = R * Cw                          # 3200 = 25*128
NST = S // 128                      # 25
HWN = H * W                         # 2048

C_IN = [512, 256, 128, 128, 128, 128, 128, 128]
C_OUT = [256, 128, 128, 128, 128, 128, 128, 128]
KS = [1, 3, 3, 3, 3, 3, 3, 3]

_CACHE = {}


def _build_base(K, kk):
    pad = kk // 2
    by = np.zeros((128, 512), np.float32)
    bx = np.zeros((128, 512), np.float32)
    hw = np.arange(HWN).reshape(4, 512)
    hh, ww = (hw // W).astype(np.float32), (hw % W).astype(np.float32)
    for c in range(4):
        for k in range(K):
            ky, kx = k // kk - pad, k % kk - pad
            by[32 * c + k] = hh[c] + ky + 8.0
            bx[32 * c + k] = ww[c] + kx + 8.0
    return np.concatenate([by, bx], axis=1)  # [128, 1024]


def _host_prep(offset_ws, deform_ws, deform_bs):
    prep = {}
    for l in range(8):
        ow = np.asarray(offset_ws[l])
        dw = np.asarray(deform_ws[l])
        db = np.asarray(deform_bs[l])
        O, C, kH, kW = dw.shape
        K = kH * kW
        NC, NO = C // 128, O // 128
        MW = 32 + K
        owt = np.zeros((128, K * NC * MW), BF16)
        ow4 = ow.reshape(K, 2, C, kH, kW)
        for k in range(K):
            ty, tx = k // kW, k % kW
            for ci in range(NC):
                sl = ow4[:, :, ci * 128:(ci + 1) * 128, ty, tx]
                c0 = (k * NC + ci) * MW
                owt[:, c0:c0 + K] = sl[:, 0].T.astype(BF16)
                owt[:, c0 + 32:c0 + 32 + K] = sl[:, 1].T.astype(BF16)
        dwt = np.zeros((128, K * NC * NO * 128), BF16)
        for k in range(K):
            ty, tx = k // kW, k % kW
            for ci in range(NC):
                for oi in range(NO):
                    r = ((k * NC + ci) * NO + oi) * 128
                    dwt[:, r:r + 128] = dw[oi * 128:(oi + 1) * 128,
                                           ci * 128:(ci + 1) * 128, ty, tx].T.astype(BF16)
        prep[f"owt{l}"] = owt
        prep[f"dwt{l}"] = dwt
        prep[f"db{l}"] = db.reshape(NO, 128).T.astype(np.float32)
    prep["base3"] = _build_base(9, 3)
    prep["base1"] = _build_base(1, 1)
    t = np.zeros((128, 4, 128), np.int16)
    t[:, 1, :] = 1
    t[:, 2, :] = Cw
    t[:, 3, :] = Cw + 1
    prep["addm"] = t.reshape(128, 512)
    return prep


def _pack_x_core(x_img):
    C = x_img.shape[0]
    pad = np.zeros((C, R, Cw), np.float32)
    pad[:, HY:HY + H, HX:HX + W] = x_img
    padf = pad.reshape(C, S).astype(BF16)
    xcf = np.ascontiguousarray(padf.reshape(C // 128, 128, S))
    xgf = np.ascontiguousarray(xcf.astype(np.float32))
    return xcf, xgf


def _memset_halo(nc, t):
    # regions overlap the interior by one row/col so the evictions that follow
    # pick up a WAW dependency on the halo zeroing (orders them for readers)
    nc.vector.memset(t[:], 0)


def _emit_layer(nc, l, pools, T, dram):
    kk = KS[l]
    K = kk * kk
    C, O = C_IN[l], C_OUT[l]
    NC, NO = C // 128, O // 128
    last = (l == 7)
    sb, psum, fp = pools["sb"], pools["psum"], pools["persist"]

    xcf, xgf = T["xcf"], T["xgf"]
    owt, dwt, db = T[f"owt{l}"], T[f"dwt{l}"], T[f"db{l}"]
    base = T["base3" if K == 9 else "base1"]
    addm = T["addm"]

    MW = 32 + K
    pBy = sb.tile([128, 512], FP32, tag="pBy", bufs=1)
    pBx = sb.tile([128, 512], FP32, tag="pBx", bufs=1)
    dy = sb.tile([128, 512], FP32, tag="dy", bufs=1)
    dx = sb.tile([128, 512], FP32, tag="dx", bufs=1)
    omy = sb.tile([128, 512], FP32, tag="omy", bufs=1)
    omx = sb.tile([128, 512], FP32, tag="omx", bufs=1)
    idx16 = sb.tile([K, 2048], I16, tag="idx16", bufs=1)
    wthin = sb.tile([128, 4 * 512], BF, tag="wthin", bufs=1)

    nc.vector.memset(pBy[:], 0)
    nc.vector.memset(pBx[:], 0)

    # ---- offset conv (psum cols: oy at 0..K-1, ox at 32..32+K-1) ----
    if l == 0:
        pso4 = [psum.tile([MW, 512], FP32, tag=f"psoff_{c}", name=f"psoff0_{c}")
                for c in range(4)]
        for ci in range(NC):
            xt = sb.tile([128, S], BF, tag="xcf0tmp", bufs=1, name=f"xcf0tmp{ci}")
            nc.sync.dma_start(xt[:], xcf[ci])
            x3 = xt[:].rearrange("p (r w) -> p r w", r=R)
            for c in range(4):
                rhs = x3[:, HY + 8 * c:HY + 8 * c + 8, HX:HX + W]
                nc.tensor.matmul(pso4[c][:], owt[:, ci * MW:(ci + 1) * MW], rhs,
                                 start=(ci == 0), stop=(ci == NC - 1))
        for c in range(4):
            nc.scalar.activation(pBy[32 * c:32 * c + K, :], pso4[c][0:K, :], AF.Copy)
            nc.scalar.activation(pBx[32 * c:32 * c + K, :], pso4[c][32:32 + K, :], AF.Copy)
    else:
        for c in range(4):
            ps_off = psum.tile([MW, 512], FP32, tag=f"psoff_{c}", name=f"psoff{l}_{c}")
            for k in range(K):
                ty, tx = k // kk - kk // 2, k % kk - kk // 2
                for ci in range(NC):
                    x3 = xcf[ci][:].rearrange("p (r w) -> p r w", r=R)
                    rhs = x3[:, HY + 8 * c + ty:HY + 8 * c + ty + 8, HX + tx:HX + tx + W]
                    c0 = (k * NC + ci) * MW
                    nc.tensor.matmul(ps_off[:], owt[:, c0:c0 + MW], rhs,
                                     start=(k == 0 and ci == 0),
                                     stop=(k == K - 1 and ci == NC - 1))
            nc.scalar.activation(pBy[32 * c:32 * c + K, :], ps_off[0:K, :], AF.Copy)
            nc.scalar.activation(pBx[32 * c:32 * c + K, :], ps_off[32:32 + K, :], AF.Copy)

    # ---- index + weight math (chunk c on partitions 32c..32c+K-1) ----
    nc.vector.tensor_add(pBy[:], pBy[:], base[:, 0:512])
    nc.vector.tensor_add(pBx[:], pBx[:], base[:, 512:1024])
    # floor via int-cast roundtrip; correction term keeps it right whether the
    # hardware cast truncates or rounds-to-nearest (values are all positive)
    for pB, dd in ((pBy, dy), (pBx, dx)):
        c16 = sb.tile([128, 512], I16, tag="c16", bufs=1)
        cf = sb.tile([128, 512], FP32, tag="cf", bufs=1)
        nc.vector.tensor_copy(c16[:], pB[:])
        nc.vector.tensor_copy(cf[:], c16[:])
        nc.vector.tensor_sub(dd[:], pB[:], cf[:])          # d0 in (-0.5, 1)
        m = sb.tile([128, 512], FP32, tag="cmask", bufs=1)
        nc.vector.tensor_scalar(m[:], dd[:], 0.0, None, OP.is_lt)
        nc.vector.tensor_sub(pB[:], cf[:], m[:])           # floor
        nc.vector.tensor_add(dd[:], dd[:], m[:])           # frac in [0, 1)
    nc.vector.tensor_scalar(omy[:], dy[:], -1.0, 1.0, OP.mult, OP.add)
    nc.vector.tensor_scalar(omx[:], dx[:], -1.0, 1.0, OP.mult, OP.add)
    nc.vector.tensor_scalar(pBy[:], pBy[:], float(Cw), float(-4 * Cw),
                            OP.mult, OP.add)
    nc.vector.tensor_add(pBy[:], pBy[:], pBx[:])      # fp32 idx00
    for c in range(4):
        nc.vector.tensor_copy(idx16[:, c * 512:(c + 1) * 512],
                              pBy[32 * c:32 * c + K, :])
    wt4 = wthin[:].rearrange("p (m s) -> p m s", m=4)
    nc.vector.tensor_mul(wt4[:, 0, :], omy[:], omx[:])
    nc.vector.tensor_mul(wt4[:, 1, :], omy[:], dx[:])
    nc.vector.tensor_mul(wt4[:, 2, :], dy[:], omx[:])
    nc.vector.tensor_mul(wt4[:, 3, :], dy[:], dx[:])

    # ---- DRAM round trips: idx wrap, weight replicate ----
    idx_d, wth_d = dram["idx"], dram["wth"]
    dst_w = AP(idx_d.tensor, idx_d.offset, [[128, K], [1, 128], [K * 128, 16]])
    nc.sync.dma_start(dst_w, idx16[:].rearrange("k (a b) -> k a b", b=16))
    for c in range(4):
        wdv = AP(wth_d.tensor, wth_d.offset + c * 512,
                 [[4 * 2048, K], [2048, 4], [1, 512]])
        nc.sync.dma_start(wdv, wthin[32 * c:32 * c + K, :]
                          .rearrange("k (m s) -> k m s", m=4))

    idx00w = sb.tile([128, K * 128], I16, tag="idx00w")
    for g in range(8):
        srcg = AP(idx_d.tensor, idx_d.offset, [[K * 128, 16], [1, K * 128]])
        nc.sync.dma_start(idx00w[16 * g:16 * (g + 1), :], srcg)

    # ---- taps: gather -> interp -> deform matmul ----
    if K > 1:
        ps_out = [[psum.tile([128, 512], FP32, tag=f"po_{c}", name=f"po{l}_{oi}_{c}")
                   for c in range(4)] for oi in range(NO)]
    def emit_gather_interp(k, ci, idxk, fat4):
        gAll = sb.tile([128, 4 * HWN], BF, tag="gAll", name=f"g{l}_{k}_{ci}")
        if l == 0:
            xg = sb.tile([128, S], FP32, tag="xgf0tmp", bufs=1,
                         name=f"xg0_{k}_{ci}")
            nc.sync.dma_start(xg[:], xgf[ci])
            xg_ap = xg[:]
        else:
            xg_ap = xgf[ci][:]
        g4 = gAll[:].rearrange("p (m n) -> p m n", m=4)
        for m in range(4):
            gtmp = sb.tile([128, HWN], FP32, tag="gtmp", bufs=1,
                           name=f"gt{l}_{k}_{ci}_{m}")
            nc.gpsimd.ap_gather(
                out_ap=gtmp[:].rearrange("p (n o) -> p n o", o=1),
                in_ap=xg_ap.rearrange("p (n o) -> p n o", o=1),
                idxs_ap=idxk[:, m * 128:(m + 1) * 128],
                channels=128, num_elems=S, d=1, num_idxs=HWN,
            )
            nc.vector.tensor_copy(g4[:, m, :], gtmp[:])
        nc.vector.tensor_mul(g4, g4, fat4)
        nc.vector.tensor_add(g4[:, 0:2, :], g4[:, 0:2, :], g4[:, 2:4, :])
        nc.vector.tensor_add(g4[:, 0, :], g4[:, 0, :], g4[:, 1, :])
        return gAll

    def emit_idxk(k):
        idxk = sb.tile([128, 512], I16, tag="idxk", name=f"idxk{l}_{k}")
        i00 = idx00w[:, k * 128:(k + 1) * 128]
        ik4 = idxk[:].rearrange("p (m s) -> p m s", m=4)
        for m, off in enumerate((0, 1, Cw, Cw + 1)):
            nc.vector.tensor_scalar(ik4[:, m, :], i00, float(off), None, OP.add)
        return idxk

    def emit_fat(k):
        fat = sb.tile([128, 4 * HWN], BF, tag="fat", bufs=1, name=f"fat{l}_{k}")
        fsrc = AP(wth_d.tensor, wth_d.offset + k * 4 * 2048, [[0, 128], [1, 4 * HWN]])
        nc.sync.dma_start(fat[:], fsrc)
        return fat[:].rearrange("p (m n) -> p m n", m=4)

    if K > 1:
        for k in range(K):
            fat4 = emit_fat(k)
            idxk = emit_idxk(k)
            for ci in range(NC):
                gAll = emit_gather_interp(k, ci, idxk, fat4)
                for oi in range(NO):
                    for c in range(4):
                        r = ((k * NC + ci) * NO + oi) * 128
                        nc.tensor.matmul(ps_out[oi][c][:], dwt[:, r:r + 128],
                                         gAll[:, c * 512:(c + 1) * 512],
                                         start=(k == 0 and ci == 0),
                                         stop=(k == K - 1 and ci == NC - 1))

    # ---- evict + next-layer tensors ----
    if K == 1:
        # L0: regather per output tile; accumulate over ci into 4 chunk psums
        xcf_n = [fp.tile([128, S], BF, tag=f"xcf_{(l + 1) % 2}_{oi}",
                         name=f"xcf{l + 1}_{oi}") for oi in range(NO)]
        for t in xcf_n:
            _memset_halo(nc, t)
        fat4 = emit_fat(0)
        idxk = emit_idxk(0)
        for oi in range(NO):
            x3 = xcf_n[oi][:].rearrange("p (r w) -> p r w", r=R)
            pso = [psum.tile([128, 512], FP32, tag=f"po_{c}", name=f"poL0_{oi}_{c}")
                   for c in range(4)]
            for ci in range(NC):
                gAll = emit_gather_interp(oi * NC + ci, ci, idxk, fat4)
                for c in range(4):
                    r = (ci * NO + oi) * 128
                    nc.tensor.matmul(pso[c][:], dwt[:, r:r + 128],
                                     gAll[:, c * 512:(c + 1) * 512],
                                     start=(ci == 0), stop=(ci == NC - 1))
            for c in range(4):
                nc.scalar.activation(x3[:, HY + 8 * c:HY + 8 * c + 8, HX:HX + W],
                                     pso[c][:], AF.Relu, bias=db[:, oi:oi + 1])
    elif last:
        out_sb = fp.tile([128, HWN], FP32, tag="out_sb", name="out_sb")
        for c in range(4):
            nc.scalar.activation(out_sb[:, c * 512:(c + 1) * 512], ps_out[0][c][:],
                                 AF.Relu, bias=db[:, 0:1])
        nc.sync.dma_start(dram["out"], out_sb[:])
        return T
    else:
        xcf_n = [fp.tile([128, S], BF, tag=f"xcf_{(l + 1) % 2}_{oi}",
                         name=f"xcf{l + 1}_{oi}") for oi in range(NO)]
        for t in xcf_n:
            _memset_halo(nc, t)
        for oi in range(NO):
            x3 = xcf_n[oi][:].rearrange("p (r w) -> p r w", r=R)
            for c in range(4):
                nc.scalar.activation(x3[:, HY + 8 * c:HY + 8 * c + 8, HX:HX + W],
                                     ps_out[oi][c][:], AF.Relu, bias=db[:, oi:oi + 1])

    xgf_n = [fp.tile([128, S], FP32, tag=f"xgf_{(l + 1) % 2}_{oi}",
                     name=f"xgf{l + 1}_{oi}") for oi in range(NO)]
    for oi in range(NO):
        nc.vector.tensor_copy(xgf_n[oi][:], xcf_n[oi][:])
    T["xcf"], T["xgf"] = xcf_n, xgf_n
    return T


def _build_program():
    nc = bacc.Bacc("TRN2", target_bir_lowering=False)
    ins = {}
    shapes = {"base3": ([128, 1024], FP32), "base1": ([128, 1024], FP32),
              "addm": ([128, 512], I16)}
    for l in range(8):
        K = KS[l] ** 2
        NC, NO = C_IN[l] // 128, C_OUT[l] // 128
        shapes[f"owt{l}"] = ([128, K * NC * (32 + K)], BF)
        shapes[f"dwt{l}"] = ([128, K * NC * NO * 128], BF)
        shapes[f"db{l}"] = ([128, NO], FP32)
    for name, (shp, dt) in shapes.items():
        ins[name] = nc.dram_tensor(name, shp, dt, kind="ExternalInput")
    ins["xcf0"] = nc.dram_tensor("xcf0", [4, 128, S], BF, kind="ExternalInput")
    ins["xgf0"] = nc.dram_tensor("xgf0", [4, 128, S], FP32, kind="ExternalInput")

    dram = {"idx": nc.dram_tensor("idx_rt", [16, 9 * 128], I16, kind="Internal")[:],
            "wth": nc.dram_tensor("wth_rt", [9, 4 * 2048], BF, kind="Internal")[:],
            "out": nc.dram_tensor("y", [128, HWN], FP32, kind="ExternalOutput")[:]}

    with tile.TileContext(nc) as tc:
        from contextlib import ExitStack
        with ExitStack() as ctx:
            pools = {"sb": ctx.enter_context(tc.tile_pool(name="sb", bufs=2)),
                     "psum": ctx.enter_context(tc.tile_pool(name="psum", bufs=1, space="PSUM")),
                     "persist": ctx.enter_context(tc.tile_pool(name="persist", bufs=1))}
            fp = pools["persist"]
            T = {}
            for name, (shp, dt) in shapes.items():
                tt = fp.tile(list(shp), dt, tag=name, name=name)
                nc.sync.dma_start(tt[:], ins[name][:])
                T[name] = tt
            T["xcf"] = [ins["xcf0"][ci] for ci in range(4)]
            T["xgf"] = [ins["xgf0"][ci] for ci in range(4)]
            nc.gpsimd.load_library(library_config.ap_gather)
            for l in range(8):
                T = _emit_layer(nc, l, pools, T, dram)
    nc.compile()
    return nc


def kernel(x, offset_ws, offset_bs, deform_ws, deform_bs):
    if "prog" not in _CACHE:
        _CACHE["prog"] = _build_program()
    nc = _CACHE["prog"]
    prep = _host_prep(offset_ws, deform_ws, deform_bs)
    xs = np.asarray(x)
    in_maps = []
    for n in range(8):
        xcf, xgf = _pack_x_core(xs[n])
        m = dict(prep)
        m["xcf0"], m["xgf0"] = xcf, xgf
        in_maps.append(m)
    import os
    trace = bool(os.environ.get("KERNEL_TRACE"))
    try:
        res = bass_utils.run_bass_kernel_spmd(nc, in_maps, core_ids=list(range(8)),
                                              trace=trace)
    except ModuleNotFoundError:
        res = bass_utils.run_bass_kernel_spmd(nc, in_maps, core_ids=list(range(8)))
    if res.exec_time_ns is not None:
        print(f"HW exec time: {res.exec_time_ns} ns")
    outs = [np.asarray(res.results[n]["y"]).reshape(128, H, W) for n in range(8)]
    return np.stack(outs).astype(np.float32)
